# revision 31
# baseline (speedup 1.0000x reference)
"""Distributed multi-head attention kernel for one TRN2 chip (8 NeuronCores).

Problem: B=4, N=2048, C=1024, H=16 heads (hd=64), fp32 in/out.
  qkv = x @ W_qkv.T ; per-head scores = q k^T * hd^-0.5 + global_bias
  attn = softmax(scores) ; out = attn @ v ; y = out @ W_proj.T + b_proj

Sharding: head-parallel attention (core c owns heads {2c, 2c+1}) + a bf16
AllToAll to token-parallel for the final projection.  Core c owns, for each
batch, token blocks [c*128, c*128+128) and [1024 + c*128, 1024+c*128+128).

v3 schedule: every batch exchanges tokens via TWO half-batch AllToAlls.
Normalization + staging for a 512-column q-chunk becomes deferred queue
pieces right after that chunk's attention completes; the half-a2a fire is
itself a queue piece ordered behind its two covering chunks, so each
collective gets roughly half a batch of flight slack before its
projection consumes it one batch later.  The attn@v ones-column trick
captures each head's softmax denominator in psum row 64; 1/den is a DVE
reciprocal_approx_fast (no ACT ln/exp), broadcast to the head's 64
partitions with a rank-1 matmul.  Warmup collectives use independent
tiny buffers so their triggers never wedge the GpSimd queue; the v ones
columns are memset once at startup (v copies skip them), so no per-kt
GpSimd work gates the attn@v matmuls.  Batch b's qkv chains are emitted
at the tail of batch b-1 (critical 8) and through batch b's own qc0
(the rest).
"""

import numpy as np
import ml_dtypes

import concourse.mybir as mybir
import concourse.tile as tile
from concourse import bacc
from concourse.bass_utils import run_bass_kernel_spmd


def _patch_act_tables():
    """This kernel uses Exp and Ln; by default the table-load pass resolves
    Exp to the `exp_and_others` set and Ln to `natural_log_exp_and_others`,
    thrashing table loads (~1.3us each) between the two.  Hide Exp/the other
    shared fns from every set except `natural_log_exp_and_others` (which has
    both) so a single table load serves the whole kernel."""
    import concourse.hw_specs as hw_specs

    if getattr(bacc, "_act_tables_patched", False):
        return
    orig = hw_specs.get_activation_tables

    def patched(module_arch):
        tables = orig(module_arch)
        keep = tables.get("natural_log_exp_and_others")
        if keep:
            e = mybir.ActivationFunctionType.Exp
            for name, fns in tables.items():
                if name != "natural_log_exp_and_others":
                    fns.discard(e)
        return tables

    bacc.get_activation_tables = patched
    bacc._act_tables_patched = True


_patch_act_tables()

F32 = mybir.dt.float32
BF16 = mybir.dt.bfloat16
BF16_NP = ml_dtypes.bfloat16

N_CORES = 8
B, N, C = 4, 2048, 1024
H = 16
HD = C // H          # 64
SCALE = HD ** -0.5
TOK = B * N          # 8192
TSLICE = TOK // N_CORES  # 1024 output tokens per core
NCT = C // 128       # 8 c-tiles
NKT = N // 128       # 16 k-tiles per batch
NQC = N // 512       # 4 q-chunks per batch
TB = TSLICE // B     # 256 tokens per (core, batch) in the final output
HB = TB // 2         # 128 tokens per (core, batch, half)
AV_SKEW = 3          # k-tiles the attn@v matmuls trail the score matmuls

_GRAPH = None


def _build():
    nc = bacc.Bacc("TRN2", target_bir_lowering=False, debug=False,
                   num_devices=N_CORES)

    xt = nc.declare_dram_parameter("xt", [C, TOK], BF16, isOutput=False)
    wq = nc.declare_dram_parameter("wq", [C, 128], BF16, isOutput=False)
    wk = nc.declare_dram_parameter("wk", [C, 128], BF16, isOutput=False)
    wv = nc.declare_dram_parameter("wv", [C, 130], BF16, isOutput=False)
    wp = nc.declare_dram_parameter("wp", [C, C], BF16, isOutput=False)
    bp = nc.declare_dram_parameter("bp", [C, 1], F32, isOutput=False)
    eb = nc.declare_dram_parameter("eb", [128, NKT, N], BF16, isOutput=False)
    out = nc.declare_dram_parameter("out", [C, TSLICE], F32, isOutput=True)

    xt_r = xt.rearrange("(ct p) t -> p ct t", p=128)
    wq_r = wq.rearrange("(ct p) f -> p ct f", p=128)
    wk_r = wk.rearrange("(ct p) f -> p ct f", p=128)
    wv_r = wv.rearrange("(ct p) f -> p ct f", p=128)
    wp_r = wp.rearrange("(ct p) o -> p ct o", p=128)
    bp_r = bp.rearrange("(ot p) one -> p ot one", p=128)

    with tile.TileContext(nc) as tc:
        with (
            tc.tile_pool(name="const", bufs=1) as cpool,
            tc.tile_pool(name="xt", bufs=1) as xpool,
            tc.tile_pool(name="qk", bufs=2) as qkpool,
            tc.tile_pool(name="vv", bufs=1) as vpool,
            tc.tile_pool(name="pp", bufs=6) as ppool,
            tc.tile_pool(name="oud", bufs=2) as oudpool,
            tc.tile_pool(name="nrm", bufs=1) as npool,
            tc.tile_pool(name="outn", bufs=1) as onpool,
            tc.tile_pool(name="rcp", bufs=1) as rpool,
            tc.tile_pool(name="fin", bufs=2) as fpool,
            tc.tile_pool(name="gat", bufs=1) as gpool,
            tc.tile_pool(name="dram", bufs=1, space="DRAM") as drpool,
            tc.tile_pool(name="ps_s", bufs=2, space="PSUM") as ps_s,
            tc.tile_pool(name="ps_o", bufs=2, space="PSUM") as ps_o,
            tc.tile_pool(name="ps_m", bufs=2, space="PSUM") as ps_m,
        ):
            # warmup collectives: absorb the one-time ENCD/rendezvous cost
            # (~60us) of the first collective behind batch 0's compute.
            # Tiny payloads; INDEPENDENT buffers so the second trigger does
            # not wait on the first warmup's completion and wedge the
            # GpSimd queue (everything behind a wedged queue head stalls).
            wz = cpool.tile([128, 16], BF16, tag="wz")
            nc.gpsimd.memset(wz[:], 0.0)
            wu_bufs = []
            for _wu in range(2):
                wu_i = drpool.tile([N_CORES, 128, 16], BF16,
                                   tag=f"wu_i{_wu}")
                wu_o = drpool.tile([N_CORES, 128, 16], BF16,
                                   tag=f"wu_o{_wu}")
                nc.sync.dma_start(wu_i[0, :, :], wz[:])
                wu_bufs.append((wu_i, wu_o))
            for wu_i, wu_o in wu_bufs:
                nc.gpsimd.collective_compute(
                    "AllToAll",
                    mybir.AluOpType.bypass,
                    replica_groups=[list(range(N_CORES))],
                    ins=[wu_i.opt()],
                    outs=[wu_o.opt()],
                )

            # ---- resident constants -------------------------------------
            wq_t = cpool.tile([128, NCT, 128], BF16, tag="wq")
            wk_t = cpool.tile([128, NCT, 128], BF16, tag="wk")
            wv_t = cpool.tile([128, NCT, 130], BF16, tag="wv")
            nc.sync.dma_start(wq_t[:], wq_r)
            nc.sync.dma_start(wk_t[:], wk_r)
            nc.sync.dma_start(wv_t[:], wv_r)
            bp_t = cpool.tile([128, NCT, 1], F32, tag="bp")
            nc.sync.dma_start(bp_t[:], bp_r)

            xt_tiles = {}

            def load_xt(bb):
                xt_t = xpool.tile([128, NCT, N], BF16, tag="xt")
                for ct in range(NCT):
                    nc.sync.dma_start(
                        xt_t[:, ct, :], xt_r[:, ct, bb * N:(bb + 1) * N]
                    )
                xt_tiles[bb] = xt_t

            # v double-buffer, managed by hand so the ones columns (attn@v
            # denominator trick) can be memset ONCE at startup and persist:
            # the per-kt v copies are strided to skip columns 64 and 129.
            # (Per-kt GpSimd memsets used to gate attn@v matmuls and stall
            # the whole pipe when the GpSimd queue was busy.)
            v_buf0 = vpool.tile([128, NKT, 130], BF16, tag="vv0")
            v_buf1 = vpool.tile([128, NKT, 130], BF16, tag="vv1")
            v_bufs = [v_buf0, v_buf1]
            for vb in v_bufs:
                nc.gpsimd.memset(vb[:, :, 64:65], 1.0)
                nc.gpsimd.memset(vb[:, :, 129:130], 1.0)

            load_xt(0)

            # absorb the one-time ACT table load (~2.7us) behind the DMAs
            scr = cpool.tile([1, 16], F32, tag="scr")
            nc.gpsimd.memset(scr[:], 1.0)
            nc.scalar.activation(scr[:], scr[:],
                                 mybir.ActivationFunctionType.Exp)
            nc.scalar.activation(scr[:], scr[:],
                                 mybir.ActivationFunctionType.Ln)

            ones_t = cpool.tile([1, 64], BF16, tag="ones")
            nc.gpsimd.memset(ones_t[:], 1.0)

            eb_ts = []
            for j in range(NKT):
                ebj = cpool.tile([128, N], BF16, tag=f"eb{j}")
                eb_ts.append(ebj)
                # sync queue, after xt(b0): keeps the startup HBM pull for
                # xt (the critical path) uncontended; eb[kt] still lands
                # well before qc0 iteration kt consumes it
                nc.sync.dma_start(ebj[:], eb[:, j, :])
            wp_t = cpool.tile([128, NCT, C], BF16, tag="wp")

            # ---- qkv chains ---------------------------------------------
            qkv_tiles = {}

            def alloc_qkv(bb):
                qT = qkpool.tile([128, N], BF16, tag="qT")
                kT = qkpool.tile([128, N], BF16, tag="kT")
                v_t = v_bufs[bb % 2]
                qkv_tiles[bb] = (qT, kT, v_t)

            def qkv_chains(bb):
                """(critical, rest) thunk lists for batch bb's qkv.
                critical = what batch bb's qc0..qc1 needs up front."""
                qT, kT, v_t = qkv_tiles[bb]
                xt_t = xt_tiles[bb]
                qk_psum = {}

                def qk_chain(dst, w_t, tcn, part=None):
                    # part=0/1 emit half the ct accumulation each, so a
                    # chain spreads over two filler slots (steadier PE/ACT
                    # overlap than one 1.7us burst)
                    if part in (None, 0):
                        pqk = ps_m.tile([128, 512], F32, tag="ps_m")
                        qk_psum[(dst is qT, tcn)] = pqk
                    else:
                        pqk = qk_psum.pop((dst is qT, tcn))
                    cts = (range(NCT) if part is None else
                           range(part * 4, part * 4 + 4))
                    for ct in cts:
                        nc.tensor.matmul(
                            pqk[:],
                            w_t[:, ct, :],
                            xt_t[:, ct, tcn * 512:(tcn + 1) * 512],
                            start=(ct == 0), stop=(ct == NCT - 1),
                        )
                    if part in (None, 1):
                        nc.vector.tensor_copy(
                            dst[:, tcn * 512:(tcn + 1) * 512], pqk[:]
                        )

                def v_chain(kt):
                    # v (+ones cols): head slices [0:65]=[v_h0|ones] and
                    # [65:130]=[v_h1|ones] put both denominators at psum
                    # row 64.  The copy skips columns 64/129 (static ones).
                    pv = ps_m.tile([128, 512], F32, tag="ps_m")
                    for ct in range(NCT):
                        nc.tensor.matmul(
                            pv[:, 0:130],
                            xt_t[:, ct, kt * 128:(kt + 1) * 128],
                            wv_t[:, ct, :],
                            start=(ct == 0), stop=(ct == NCT - 1),
                        )
                    nc.vector.tensor_copy(
                        v_t[:, kt, :].rearrange("p (h c) -> p h c", h=2)
                        [:, :, 0:64],
                        pv[:, 0:130].rearrange("p (h c) -> p h c", h=2)
                        [:, :, 0:64],
                    )

                def qk2(dst, w_t, tcn):
                    # a chain as two adjacent half pieces (keep adjacent in
                    # the queue: they share one ps_m ring slot)
                    return [lambda: qk_chain(dst, w_t, tcn, part=0),
                            lambda: qk_chain(dst, w_t, tcn, part=1)]

                crit = []
                for tcn in range(4):
                    crit += qk2(kT, wk_t, tcn)
                crit += qk2(qT, wq_t, 0)
                crit.append(lambda: v_chain(0))
                crit.append(lambda: v_chain(1))
                crit += qk2(qT, wq_t, 1)
                rest = [lambda k=kt: v_chain(k) for kt in range(2, 10)]
                rest += qk2(qT, wq_t, 2)
                rest += [lambda k=kt: v_chain(k) for kt in range(10, NKT)]
                rest += qk2(qT, wq_t, 3)
                return crit, rest

            # ---- deferred normalization + AllToAll staging --------------
            # Per-qc clusters: right after a 512-column q-chunk of batch
            # bb's attention output lands in oud, deferred pieces compute
            # 1/den on the DVE (reciprocal_approx_fast -- no ACT ln/exp),
            # broadcast it with a rank-1 matmul, normalize, and stage that
            # chunk of the half-batch AllToAll.  The half-a2a fire is
            # itself a queue piece, so it triggers as soon as the covering
            # clusters drain (h0 after qc1, h1 after qc3) -- giving each
            # collective a ~full-half-batch of flight slack.
            ouds = {}
            cl_state = {}

            def alloc_cluster(bb):
                rcp0 = rpool.tile([1, N], BF16, tag="rcp0")
                rcp1 = rpool.tile([1, N], BF16, tag="rcp1")
                bc0 = npool.tile([64, N], BF16, tag="bc0")
                bc1 = npool.tile([64, N], BF16, tag="bc1")
                on0 = onpool.tile([64, N], BF16, tag="on0")
                on1 = onpool.tile([64, N], BF16, tag="on1")
                a2a = []
                for hf in range(2):
                    a2a_i = drpool.tile([N_CORES, 128, HB], BF16,
                                        tag=f"a2ai{bb}h{hf}")
                    a2a_o = drpool.tile([N_CORES, 128, HB], BF16,
                                        tag=f"a2ao{bb}h{hf}")
                    a2a.append((a2a_i, a2a_o))
                cl_state[bb] = {"rcp": [rcp0, rcp1], "bc": [bc0, bc1],
                                "on": [on0, on1], "a2a": a2a}

            def qc_cluster(bb, qc):
                """Deferred pieces normalizing + staging batch bb's
                attention-output columns [qc*512, qc*512+512)."""
                st = cl_state[bb]
                oud2 = ouds[bb]
                lo = qc * 512
                half = qc // 2
                a2a_i = st["a2a"][half][0]
                pieces = []

                def t_rcp(h):
                    # 1/den via DVE fast reciprocal (f32 in/out)
                    denf = rpool.tile([1, 512], F32, tag="denf")
                    recf = rpool.tile([1, 512], F32, tag="recf")
                    nc.vector.tensor_copy(
                        denf[:], oud2[h][64:65, lo:lo + 512])
                    nc.vector.reciprocal_approx_fast(recf[:], denf[:])
                    nc.vector.tensor_copy(
                        st["rcp"][h][0:1, lo:lo + 512], recf[:])

                def t_pb(h):
                    # broadcast 1/den over the head's 64 partitions
                    pb = ps_m.tile([64, 512], F32, tag="ps_m")
                    nc.tensor.matmul(
                        pb[:], ones_t[:], st["rcp"][h][0:1, lo:lo + 512],
                        start=True, stop=True,
                    )
                    nc.vector.tensor_copy(st["bc"][h][:, lo:lo + 512], pb[:])

                def t_mul(h):
                    nc.vector.tensor_mul(
                        st["on"][h][:, lo:lo + 512],
                        oud2[h][0:64, lo:lo + 512],
                        st["bc"][h][:, lo:lo + 512],
                    )

                def t_dma():
                    j0 = (lo % 1024) // HB
                    for j in range(j0, j0 + 4):
                        c0 = half * 1024 + j * HB
                        nc.sync.dma_start(
                            a2a_i[j, 0:64, :], st["on"][0][:, c0:c0 + HB])
                        nc.sync.dma_start(
                            a2a_i[j, 64:128, :], st["on"][1][:, c0:c0 + HB])

                for h in range(2):
                    pieces.append(lambda h=h: t_rcp(h))
                for h in range(2):
                    pieces.append(lambda h=h: t_pb(h))
                for h in range(2):
                    pieces.append(lambda h=h: t_mul(h))
                pieces.append(t_dma)
                return pieces

            def fire_a2a(a2a_i, a2a_o):
                nc.gpsimd.collective_compute(
                    "AllToAll",
                    mybir.AluOpType.bypass,
                    replica_groups=[list(range(N_CORES))],
                    ins=[a2a_i.opt()],
                    outs=[a2a_o.opt()],
                )

            # ---- projection ---------------------------------------------
            def proj_pieces(halves, ncols, col0):
                """halves: list of (a2a_o, gat column offset, width)."""
                pieces = []
                gat = gpool.tile([128, NCT, TB], BF16, tag="gat")

                def t_gather():
                    for a2a_o_, g0, w in halves:
                        for ct in range(NCT):
                            nc.sync.dma_start(gat[:, ct, g0:g0 + w],
                                              a2a_o_[ct, :, :])

                pieces.append(t_gather)

                def t_ot(ot):
                    pf = ps_m.tile([128, ncols], F32, tag="ps_m")
                    for ct in range(NCT):
                        nc.tensor.matmul(
                            pf[:],
                            wp_t[:, ct, ot * 128:(ot + 1) * 128],
                            gat[:, ct, 0:ncols],
                            start=(ct == 0), stop=(ct == NCT - 1),
                        )
                    fin = fpool.tile([128, ncols], F32, tag=f"fin{ncols}")
                    nc.vector.tensor_scalar_add(fin[:], pf[:], bp_t[:, ot, :])
                    nc.sync.dma_start(
                        out[ot * 128:(ot + 1) * 128, col0:col0 + ncols],
                        fin[:],
                    )

                for ot in range(NCT):
                    pieces.append(lambda o=ot: t_ot(o))
                return pieces

            # ---- batch 0 prologue ---------------------------------------
            alloc_qkv(0)
            crit0, rest0 = qkv_chains(0)
            for chz in crit0:
                chz()
            for ct in range(NCT):
                nc.gpsimd.dma_start(wp_t[:, ct, :], wp_r[:, ct, :])

            dq = []          # deferred work: (kind, thunk)
            qkv_rest = {}
            dq += [("chain", t) for t in rest0]
            pending_projs = {}    # b -> list of proj pieces

            def drain(n):
                for _ in range(n):
                    if not dq:
                        return
                    _, t = dq.pop(0)
                    t()

            # ---- per-batch attention ------------------------------------
            for b in range(B):
                qT, kT, v_t = qkv_tiles.pop(b)
                xt_tiles.pop(b)
                oud0 = oudpool.tile([65, N], BF16, tag="oud0")
                oud1 = oudpool.tile([65, N], BF16, tag="oud1")
                ouds[b] = (oud0, oud1)
                last = b == B - 1
                alloc_cluster(b)

                for qc in range(NQC):
                    q0 = qc * 512
                    po0 = ps_o.tile([65, 512], F32, tag="ps_o")
                    po1 = ps_o.tile([65, 512], F32, tag="ps_o")
                    po = [po0, po1]
                    pend = []
                    for kt in range(NKT):
                        ps = ps_s.tile([128, 1024], F32, tag="ps_s")
                        for h in range(2):
                            nc.tensor.matmul(
                                ps[:, h * 512:(h + 1) * 512],
                                kT[h * 64:h * 64 + 64,
                                   kt * 128:(kt + 1) * 128],
                                qT[h * 64:h * 64 + 64, q0:q0 + 512],
                                start=True, stop=True,
                            )
                        if len(pend) >= AV_SKEW:
                            pkt, ppw = pend.pop(0)
                            for h in range(2):
                                nc.tensor.matmul(
                                    po[h][:],
                                    v_t[:, pkt, h * 65:h * 65 + 65],
                                    ppw[:, h * 512:(h + 1) * 512],
                                    start=(pkt == 0), stop=False,
                                )
                        pexp = ppool.tile([128, 1024], BF16, tag="pp")
                        nc.scalar.activation(
                            pexp[:], ps[:],
                            mybir.ActivationFunctionType.Exp, scale=SCALE,
                        )
                        pw = ppool.tile([128, 1024], BF16, tag="pp")
                        ebb = (eb_ts[kt][:, q0:q0 + 512]
                               .unsqueeze(1).to_broadcast([128, 2, 512]))
                        nc.vector.tensor_mul(
                            pw[:].rearrange("p (h q) -> p h q", h=2),
                            pexp[:].rearrange("p (h q) -> p h q", h=2),
                            ebb,
                        )
                        pend.append((kt, pw))
                        # exactly one filler piece per kt iteration: the
                        # kt pipeline is ACT(exp)-bound at ~1.1us/kt while
                        # the PE's own scores+av take ~750ns, so a steady
                        # ~1 piece/iter keeps the PE dense (no HAM
                        # re-throttle) without starving the exp stream
                        drain(2 if len(dq) > 24 else 1)
                    for pkt, ppw in pend:
                        for h in range(2):
                            nc.tensor.matmul(
                                po[h][:],
                                v_t[:, pkt, h * 65:h * 65 + 65],
                                ppw[:, h * 512:(h + 1) * 512],
                                start=False, stop=(pkt == NKT - 1),
                            )
                    nc.scalar.copy(oud0[:, q0:q0 + 512], po0[:])
                    nc.scalar.copy(oud1[:, q0:q0 + 512], po1[:])

                    # ---- end-of-qc hooks --------------------------------
                    dq += [("cluster", t) for t in qc_cluster(b, qc)]
                    st = cl_state[b]
                    if qc == 1:
                        # fire the first-half a2a once its clusters drain
                        dq.append(("fire",
                                   lambda st=st: fire_a2a(*st["a2a"][0])))
                    if qc == 0:
                        if b + 1 < B:
                            # all queued chain pieces read the current xt
                            # buffers; they must be emitted before the xt
                            # reload below or they'd read b+1's data
                            while any(k == "chain" for k, _ in dq):
                                drain(1)
                            load_xt(b + 1)
                    elif qc == 1 and not last:
                        alloc_qkv(b + 1)
                        critn, restn = qkv_chains(b + 1)
                        qkv_rest[b + 1] = restn
                        dq += [("chain", t) for t in critn]
                    elif qc == 2 and not last:
                        dq += [("chain", t)
                               for t in qkv_rest.pop(b + 1)]
                    elif qc == 2 and last:
                        dq += [("proj", t)
                               for t in pending_projs.pop(b - 1)]
                        # the last batch's first-half a2a lands during
                        # qc3: its projection can drain there too
                        dq += [("proj", t) for t in proj_pieces(
                            [(cl_state[b]["a2a"][0][1], 0, HB)],
                            HB, b * TB)]

                # ---- end-of-batch ---------------------------------------
                st = cl_state.pop(b)
                dq.append(("fire", lambda st=st: fire_a2a(*st["a2a"][1])))
                if not last:
                    # b-1's projection drains during b+1's qc0: that zone
                    # is exp-bound with no chain filler left, and proj
                    # pieces read only gat/wp (safe past the xt reload)
                    if b >= 1:
                        dq += [("proj", t)
                               for t in pending_projs.pop(b - 1)]
                    pending_projs[b] = proj_pieces(
                        [(st["a2a"][0][1], 0, HB), (st["a2a"][1][1], HB, HB)],
                        TB, b * TB)
                else:
                    # drain leftovers (finishes the qc3 cluster + h1
                    # fire), then keep the PE warm across the final
                    # collective flight with throwaway matmuls so HAM
                    # stays at 8/8 and the last projection runs fast
                    drain(len(dq))

                    def warm(n):
                        for _ in range(n):
                            pd = ps_m.tile([128, 512], F32, tag="ps_m")
                            nc.tensor.matmul(
                                pd[:], wq_t[:, 0, :], wp_t[:, 0, 0:512],
                                start=True, stop=True,
                            )

                    warm(90)
                    # interleave warm matmuls with the projection pieces
                    # so the PE never goes HAM-sparse before the end
                    for t in proj_pieces([(st["a2a"][1][1], 0, HB)],
                                         HB, b * TB + HB):
                        t()
                        warm(4)

    nc.compile()
    return nc


def _graph():
    global _GRAPH
    if _GRAPH is None:
        _GRAPH = _build()
    return _GRAPH


def _prep_inputs(x, W_qkv, W_proj, b_proj, global_bias):
    x = np.asarray(x, dtype=np.float32)
    W_qkv = np.asarray(W_qkv, dtype=np.float32)
    W_proj = np.asarray(W_proj, dtype=np.float32)
    b_proj = np.asarray(b_proj, dtype=np.float32)
    global_bias = np.asarray(global_bias, dtype=np.float32)

    xt = np.ascontiguousarray(x.reshape(TOK, C).T).astype(BF16_NP)
    wpv = np.ascontiguousarray(W_proj.T).astype(BF16_NP)
    bpv = np.ascontiguousarray(b_proj[:, None])
    ebt = np.exp(global_bias).T  # [k, q]
    ebp = np.ascontiguousarray(
        ebt.reshape(NKT, 128, N).transpose(1, 0, 2)
    ).astype(BF16_NP)

    in_maps = []
    for c in range(N_CORES):
        r0 = c * 128
        wq_c = np.ascontiguousarray(W_qkv[r0:r0 + 128, :].T).astype(BF16_NP)
        wk_c = np.ascontiguousarray(W_qkv[C + r0:C + r0 + 128, :].T).astype(BF16_NP)
        vt = W_qkv[2 * C + r0:2 * C + r0 + 128, :].T  # [C, 128]
        wv_c = np.zeros((C, 130), dtype=np.float32)
        wv_c[:, 0:64] = vt[:, 0:64]
        wv_c[:, 65:129] = vt[:, 64:128]
        in_maps.append({
            "xt": xt,
            "wq": wq_c,
            "wk": wk_c,
            "wv": wv_c.astype(BF16_NP),
            "wp": wpv,
            "bp": bpv,
            "eb": ebp,
        })
    return in_maps


def _assemble(results):
    full = np.empty((TOK, C), dtype=np.float32)
    for c in range(N_CORES):
        o = results[c]["out"].T  # [TSLICE tokens, C]
        for b in range(B):
            full[b * N + c * HB:b * N + (c + 1) * HB, :] = (
                o[b * TB:b * TB + HB, :]
            )
            full[b * N + 1024 + c * HB:b * N + 1024 + (c + 1) * HB, :] = (
                o[b * TB + HB:(b + 1) * TB, :]
            )
    return full.reshape(B, N, C)


def kernel(x, W_qkv, W_proj, b_proj, global_bias):
    nc = _graph()
    in_maps = _prep_inputs(x, W_qkv, W_proj, b_proj, global_bias)
    res = run_bass_kernel_spmd(nc, in_maps, core_ids=list(range(N_CORES)))
    return _assemble(res.results)


def run_profiled(x, W_qkv, W_proj, b_proj, global_bias, **trace_kwargs):
    """Like kernel() but with NTFF profiling; returns (output, results)."""
    nc = _graph()
    in_maps = _prep_inputs(x, W_qkv, W_proj, b_proj, global_bias)
    res = run_bass_kernel_spmd(
        nc, in_maps, core_ids=list(range(N_CORES)), trace=True, **trace_kwargs
    )
    return _assemble(res.results), res



# revision 32
# speedup vs baseline: 1.0210x; 1.0210x over previous
"""Distributed multi-head attention kernel for one TRN2 chip (8 NeuronCores).

Problem: B=4, N=2048, C=1024, H=16 heads (hd=64), fp32 in/out.
  qkv = x @ W_qkv.T ; per-head scores = q k^T * hd^-0.5 + global_bias
  attn = softmax(scores) ; out = attn @ v ; y = out @ W_proj.T + b_proj

Sharding: head-parallel attention (core c owns heads {2c, 2c+1}) + a bf16
AllToAll to token-parallel for the final projection.  Core c owns, for each
batch, token blocks [c*128, c*128+128) and [1024 + c*128, 1024+c*128+128).

v3 schedule: every batch exchanges tokens via TWO half-batch AllToAlls.
Normalization + staging for a 512-column q-chunk becomes deferred queue
pieces right after that chunk's attention completes; the half-a2a fire is
itself a queue piece ordered behind its two covering chunks, so each
collective gets roughly half a batch of flight slack before its
projection consumes it one batch later.  The attn@v ones-column trick
captures each head's softmax denominator in psum row 64; 1/den is a DVE
reciprocal_approx_fast (no ACT ln/exp), broadcast to the head's 64
partitions with a rank-1 matmul.  Warmup collectives use independent
tiny buffers so their triggers never wedge the GpSimd queue; the v ones
columns are memset once at startup (v copies skip them), so no per-kt
GpSimd work gates the attn@v matmuls.  Batch b's qkv chains are emitted
at the tail of batch b-1 (critical 8) and through batch b's own qc0
(the rest).
"""

import numpy as np
import ml_dtypes

import concourse.mybir as mybir
import concourse.tile as tile
from concourse import bacc
from concourse.bass_utils import run_bass_kernel_spmd


def _patch_act_tables():
    """This kernel uses Exp and Ln; by default the table-load pass resolves
    Exp to the `exp_and_others` set and Ln to `natural_log_exp_and_others`,
    thrashing table loads (~1.3us each) between the two.  Hide Exp/the other
    shared fns from every set except `natural_log_exp_and_others` (which has
    both) so a single table load serves the whole kernel."""
    import concourse.hw_specs as hw_specs

    if getattr(bacc, "_act_tables_patched", False):
        return
    orig = hw_specs.get_activation_tables

    def patched(module_arch):
        tables = orig(module_arch)
        keep = tables.get("natural_log_exp_and_others")
        if keep:
            e = mybir.ActivationFunctionType.Exp
            for name, fns in tables.items():
                if name != "natural_log_exp_and_others":
                    fns.discard(e)
        return tables

    bacc.get_activation_tables = patched
    bacc._act_tables_patched = True


_patch_act_tables()

F32 = mybir.dt.float32
BF16 = mybir.dt.bfloat16
BF16_NP = ml_dtypes.bfloat16

N_CORES = 8
B, N, C = 4, 2048, 1024
H = 16
HD = C // H          # 64
SCALE = HD ** -0.5
TOK = B * N          # 8192
TSLICE = TOK // N_CORES  # 1024 output tokens per core
NCT = C // 128       # 8 c-tiles
NKT = N // 128       # 16 k-tiles per batch
NQC = N // 512       # 4 q-chunks per batch
TB = TSLICE // B     # 256 tokens per (core, batch) in the final output
HB = TB // 2         # 128 tokens per (core, batch, half)
AV_SKEW = 3          # k-tiles the attn@v matmuls trail the score matmuls

_GRAPH = None


def _build():
    nc = bacc.Bacc("TRN2", target_bir_lowering=False, debug=False,
                   num_devices=N_CORES)

    xt = nc.declare_dram_parameter("xt", [C, TOK], BF16, isOutput=False)
    wq = nc.declare_dram_parameter("wq", [C, 128], BF16, isOutput=False)
    wk = nc.declare_dram_parameter("wk", [C, 128], BF16, isOutput=False)
    wv = nc.declare_dram_parameter("wv", [C, 130], BF16, isOutput=False)
    wp = nc.declare_dram_parameter("wp", [C, C], BF16, isOutput=False)
    bp = nc.declare_dram_parameter("bp", [C, 1], F32, isOutput=False)
    eb = nc.declare_dram_parameter("eb", [128, NKT, N], BF16, isOutput=False)
    out = nc.declare_dram_parameter("out", [C, TSLICE], F32, isOutput=True)

    xt_r = xt.rearrange("(ct p) t -> p ct t", p=128)
    wq_r = wq.rearrange("(ct p) f -> p ct f", p=128)
    wk_r = wk.rearrange("(ct p) f -> p ct f", p=128)
    wv_r = wv.rearrange("(ct p) f -> p ct f", p=128)
    wp_r = wp.rearrange("(ct p) o -> p ct o", p=128)
    bp_r = bp.rearrange("(ot p) one -> p ot one", p=128)

    with tile.TileContext(nc) as tc:
        with (
            tc.tile_pool(name="const", bufs=1) as cpool,
            tc.tile_pool(name="xt", bufs=1) as xpool,
            tc.tile_pool(name="qk", bufs=2) as qkpool,
            tc.tile_pool(name="vv", bufs=1) as vpool,
            tc.tile_pool(name="pp", bufs=6) as ppool,
            tc.tile_pool(name="oud", bufs=2) as oudpool,
            tc.tile_pool(name="nrm", bufs=1) as npool,
            tc.tile_pool(name="outn", bufs=1) as onpool,
            tc.tile_pool(name="rcp", bufs=1) as rpool,
            tc.tile_pool(name="fin", bufs=2) as fpool,
            tc.tile_pool(name="gat", bufs=1) as gpool,
            tc.tile_pool(name="dram", bufs=1, space="DRAM") as drpool,
            tc.tile_pool(name="ps_s", bufs=2, space="PSUM") as ps_s,
            tc.tile_pool(name="ps_o", bufs=2, space="PSUM") as ps_o,
            tc.tile_pool(name="ps_m", bufs=2, space="PSUM") as ps_m,
        ):
            # warmup collectives: absorb the one-time ENCD/rendezvous cost
            # (~60us) of the first collective behind batch 0's compute.
            # Tiny payloads; INDEPENDENT buffers so the second trigger does
            # not wait on the first warmup's completion and wedge the
            # GpSimd queue (everything behind a wedged queue head stalls).
            wz = cpool.tile([128, 16], BF16, tag="wz")
            nc.gpsimd.memset(wz[:], 0.0)
            wu_bufs = []
            for _wu in range(2):
                wu_i = drpool.tile([N_CORES, 128, 16], BF16,
                                   tag=f"wu_i{_wu}")
                wu_o = drpool.tile([N_CORES, 128, 16], BF16,
                                   tag=f"wu_o{_wu}")
                nc.sync.dma_start(wu_i[0, :, :], wz[:])
                wu_bufs.append((wu_i, wu_o))
            for wu_i, wu_o in wu_bufs:
                nc.gpsimd.collective_compute(
                    "AllToAll",
                    mybir.AluOpType.bypass,
                    replica_groups=[list(range(N_CORES))],
                    ins=[wu_i.opt()],
                    outs=[wu_o.opt()],
                )

            # ---- resident constants -------------------------------------
            wq_t = cpool.tile([128, NCT, 128], BF16, tag="wq")
            wk_t = cpool.tile([128, NCT, 128], BF16, tag="wk")
            wv_t = cpool.tile([128, NCT, 130], BF16, tag="wv")
            nc.sync.dma_start(wq_t[:], wq_r)
            nc.sync.dma_start(wk_t[:], wk_r)
            nc.sync.dma_start(wv_t[:], wv_r)
            bp_t = cpool.tile([128, NCT, 1], F32, tag="bp")
            nc.sync.dma_start(bp_t[:], bp_r)

            xt_tiles = {}

            def load_xt(bb):
                xt_t = xpool.tile([128, NCT, N], BF16, tag="xt")
                for ct in range(NCT):
                    nc.sync.dma_start(
                        xt_t[:, ct, :], xt_r[:, ct, bb * N:(bb + 1) * N]
                    )
                xt_tiles[bb] = xt_t

            # v double-buffer, managed by hand so the ones columns (attn@v
            # denominator trick) can be memset ONCE at startup and persist:
            # the per-kt v copies are strided to skip columns 64 and 129.
            # (Per-kt GpSimd memsets used to gate attn@v matmuls and stall
            # the whole pipe when the GpSimd queue was busy.)
            v_buf0 = vpool.tile([128, NKT, 130], BF16, tag="vv0")
            v_buf1 = vpool.tile([128, NKT, 130], BF16, tag="vv1")
            v_bufs = [v_buf0, v_buf1]
            for vb in v_bufs:
                nc.gpsimd.memset(vb[:, :, 64:65], 1.0)
                nc.gpsimd.memset(vb[:, :, 129:130], 1.0)

            load_xt(0)

            # absorb the one-time ACT table load (~2.7us) behind the DMAs
            scr = cpool.tile([1, 16], F32, tag="scr")
            nc.gpsimd.memset(scr[:], 1.0)
            nc.scalar.activation(scr[:], scr[:],
                                 mybir.ActivationFunctionType.Exp)
            nc.scalar.activation(scr[:], scr[:],
                                 mybir.ActivationFunctionType.Ln)

            ones_t = cpool.tile([1, 64], BF16, tag="ones")
            nc.gpsimd.memset(ones_t[:], 1.0)

            eb_ts = []
            for j in range(NKT):
                ebj = cpool.tile([128, N], BF16, tag=f"eb{j}")
                eb_ts.append(ebj)
                # sync queue, after xt(b0): keeps the startup HBM pull for
                # xt (the critical path) uncontended; eb[kt] still lands
                # well before qc0 iteration kt consumes it
                nc.sync.dma_start(ebj[:], eb[:, j, :])
            wp_t = cpool.tile([128, NCT, C], BF16, tag="wp")

            # ---- qkv chains ---------------------------------------------
            qkv_tiles = {}

            def alloc_qkv(bb):
                qT = qkpool.tile([128, N], BF16, tag="qT")
                kT = qkpool.tile([128, N], BF16, tag="kT")
                v_t = v_bufs[bb % 2]
                qkv_tiles[bb] = (qT, kT, v_t)

            def qkv_chains(bb):
                """(critical, rest) thunk lists for batch bb's qkv.
                critical = what batch bb's qc0..qc1 needs up front."""
                qT, kT, v_t = qkv_tiles[bb]
                xt_t = xt_tiles[bb]
                qk_psum = {}

                def qk_chain(dst, w_t, tcn, part=None):
                    # part=0/1 emit half the ct accumulation each, so a
                    # chain spreads over two filler slots (steadier PE/ACT
                    # overlap than one 1.7us burst)
                    if part in (None, 0):
                        pqk = ps_m.tile([128, 512], F32, tag="ps_m")
                        qk_psum[(dst is qT, tcn)] = pqk
                    else:
                        pqk = qk_psum.pop((dst is qT, tcn))
                    cts = (range(NCT) if part is None else
                           range(part * 4, part * 4 + 4))
                    for ct in cts:
                        nc.tensor.matmul(
                            pqk[:],
                            w_t[:, ct, :],
                            xt_t[:, ct, tcn * 512:(tcn + 1) * 512],
                            start=(ct == 0), stop=(ct == NCT - 1),
                        )
                    if part in (None, 1):
                        nc.vector.tensor_copy(
                            dst[:, tcn * 512:(tcn + 1) * 512], pqk[:]
                        )

                def v_chain(kt):
                    # v (+ones cols): head slices [0:65]=[v_h0|ones] and
                    # [65:130]=[v_h1|ones] put both denominators at psum
                    # row 64.  The copy skips columns 64/129 (static ones).
                    pv = ps_m.tile([128, 512], F32, tag="ps_m")
                    for ct in range(NCT):
                        nc.tensor.matmul(
                            pv[:, 0:130],
                            xt_t[:, ct, kt * 128:(kt + 1) * 128],
                            wv_t[:, ct, :],
                            start=(ct == 0), stop=(ct == NCT - 1),
                        )
                    nc.vector.tensor_copy(
                        v_t[:, kt, :].rearrange("p (h c) -> p h c", h=2)
                        [:, :, 0:64],
                        pv[:, 0:130].rearrange("p (h c) -> p h c", h=2)
                        [:, :, 0:64],
                    )

                def qk2(dst, w_t, tcn):
                    # a chain as two adjacent half pieces (keep adjacent in
                    # the queue: they share one ps_m ring slot)
                    return [lambda: qk_chain(dst, w_t, tcn, part=0),
                            lambda: qk_chain(dst, w_t, tcn, part=1)]

                crit = []
                for tcn in range(4):
                    crit += qk2(kT, wk_t, tcn)
                crit += qk2(qT, wq_t, 0)
                crit.append(lambda: v_chain(0))
                crit.append(lambda: v_chain(1))
                crit += qk2(qT, wq_t, 1)
                rest = [lambda k=kt: v_chain(k) for kt in range(2, 10)]
                rest += qk2(qT, wq_t, 2)
                rest += [lambda k=kt: v_chain(k) for kt in range(10, NKT)]
                rest += qk2(qT, wq_t, 3)
                return crit, rest

            # ---- deferred normalization + AllToAll staging --------------
            # Per-qc clusters: right after a 512-column q-chunk of batch
            # bb's attention output lands in oud, deferred pieces compute
            # 1/den on the DVE (reciprocal_approx_fast -- no ACT ln/exp),
            # broadcast it with a rank-1 matmul, normalize, and stage that
            # chunk of the half-batch AllToAll.  The half-a2a fire is
            # itself a queue piece, so it triggers as soon as the covering
            # clusters drain (h0 after qc1, h1 after qc3) -- giving each
            # collective a ~full-half-batch of flight slack.
            ouds = {}
            cl_state = {}

            def alloc_cluster(bb):
                rcp0 = rpool.tile([1, N], BF16, tag="rcp0")
                rcp1 = rpool.tile([1, N], BF16, tag="rcp1")
                bc0 = npool.tile([64, N], BF16, tag="bc0")
                bc1 = npool.tile([64, N], BF16, tag="bc1")
                on0 = onpool.tile([64, N], BF16, tag="on0")
                on1 = onpool.tile([64, N], BF16, tag="on1")
                a2a = []
                for hf in range(2):
                    a2a_i = drpool.tile([N_CORES, 128, HB], BF16,
                                        tag=f"a2ai{bb}h{hf}")
                    a2a_o = drpool.tile([N_CORES, 128, HB], BF16,
                                        tag=f"a2ao{bb}h{hf}")
                    a2a.append((a2a_i, a2a_o))
                cl_state[bb] = {"rcp": [rcp0, rcp1], "bc": [bc0, bc1],
                                "on": [on0, on1], "a2a": a2a}

            def qc_cluster(bb, qc):
                """Deferred pieces normalizing + staging batch bb's
                attention-output columns [qc*512, qc*512+512)."""
                st = cl_state[bb]
                oud2 = ouds[bb]
                lo = qc * 512
                half = qc // 2
                a2a_i = st["a2a"][half][0]
                pieces = []

                def t_rcp(h):
                    # 1/den via DVE fast reciprocal (f32 in/out)
                    denf = rpool.tile([1, 512], F32, tag="denf")
                    recf = rpool.tile([1, 512], F32, tag="recf")
                    nc.vector.tensor_copy(
                        denf[:], oud2[h][64:65, lo:lo + 512])
                    nc.vector.reciprocal_approx_fast(recf[:], denf[:])
                    nc.vector.tensor_copy(
                        st["rcp"][h][0:1, lo:lo + 512], recf[:])

                def t_pb(h):
                    # broadcast 1/den over the head's 64 partitions
                    pb = ps_m.tile([64, 512], F32, tag="ps_m")
                    nc.tensor.matmul(
                        pb[:], ones_t[:], st["rcp"][h][0:1, lo:lo + 512],
                        start=True, stop=True,
                    )
                    nc.vector.tensor_copy(st["bc"][h][:, lo:lo + 512], pb[:])

                def t_mul(h):
                    nc.vector.tensor_mul(
                        st["on"][h][:, lo:lo + 512],
                        oud2[h][0:64, lo:lo + 512],
                        st["bc"][h][:, lo:lo + 512],
                    )

                def t_dma():
                    j0 = (lo % 1024) // HB
                    for j in range(j0, j0 + 4):
                        c0 = half * 1024 + j * HB
                        nc.sync.dma_start(
                            a2a_i[j, 0:64, :], st["on"][0][:, c0:c0 + HB])
                        nc.sync.dma_start(
                            a2a_i[j, 64:128, :], st["on"][1][:, c0:c0 + HB])

                for h in range(2):
                    pieces.append(lambda h=h: t_rcp(h))
                for h in range(2):
                    pieces.append(lambda h=h: t_pb(h))
                for h in range(2):
                    pieces.append(lambda h=h: t_mul(h))
                pieces.append(t_dma)
                return pieces

            def fire_a2a(a2a_i, a2a_o):
                nc.gpsimd.collective_compute(
                    "AllToAll",
                    mybir.AluOpType.bypass,
                    replica_groups=[list(range(N_CORES))],
                    ins=[a2a_i.opt()],
                    outs=[a2a_o.opt()],
                )

            # ---- projection ---------------------------------------------
            def proj_pieces(halves, ncols, col0):
                """halves: list of (a2a_o, gat column offset, width)."""
                pieces = []
                gat = gpool.tile([128, NCT, TB], BF16, tag="gat")

                def t_gather():
                    for a2a_o_, g0, w in halves:
                        for ct in range(NCT):
                            nc.sync.dma_start(gat[:, ct, g0:g0 + w],
                                              a2a_o_[ct, :, :])

                pieces.append(t_gather)

                def t_ot(ot):
                    pf = ps_m.tile([128, ncols], F32, tag="ps_m")
                    for ct in range(NCT):
                        nc.tensor.matmul(
                            pf[:],
                            wp_t[:, ct, ot * 128:(ot + 1) * 128],
                            gat[:, ct, 0:ncols],
                            start=(ct == 0), stop=(ct == NCT - 1),
                        )
                    fin = fpool.tile([128, ncols], F32, tag=f"fin{ncols}")
                    nc.vector.tensor_scalar_add(fin[:], pf[:], bp_t[:, ot, :])
                    nc.sync.dma_start(
                        out[ot * 128:(ot + 1) * 128, col0:col0 + ncols],
                        fin[:],
                    )

                for ot in range(NCT):
                    pieces.append(lambda o=ot: t_ot(o))
                return pieces

            # ---- batch 0 prologue ---------------------------------------
            alloc_qkv(0)
            crit0, rest0 = qkv_chains(0)
            for chz in crit0:
                chz()
            for ct in range(NCT):
                nc.gpsimd.dma_start(wp_t[:, ct, :], wp_r[:, ct, :])

            dq = []          # deferred work: (kind, thunk)
            qkv_rest = {}
            dq += [("chain", t) for t in rest0]
            pending_projs = {}    # b -> list of proj pieces

            def drain(n):
                for _ in range(n):
                    if not dq:
                        return
                    _, t = dq.pop(0)
                    t()

            # ---- per-batch attention ------------------------------------
            for b in range(B):
                qT, kT, v_t = qkv_tiles.pop(b)
                xt_tiles.pop(b)
                oud0 = oudpool.tile([65, N], BF16, tag="oud0")
                oud1 = oudpool.tile([65, N], BF16, tag="oud1")
                ouds[b] = (oud0, oud1)
                last = b == B - 1
                alloc_cluster(b)

                for qc in range(NQC):
                    q0 = qc * 512
                    po0 = ps_o.tile([65, 512], F32, tag="ps_o")
                    po1 = ps_o.tile([65, 512], F32, tag="ps_o")
                    po = [po0, po1]
                    pend = []
                    for kt in range(NKT):
                        ps = ps_s.tile([128, 1024], F32, tag="ps_s")
                        for h in range(2):
                            nc.tensor.matmul(
                                ps[:, h * 512:(h + 1) * 512],
                                kT[h * 64:h * 64 + 64,
                                   kt * 128:(kt + 1) * 128],
                                qT[h * 64:h * 64 + 64, q0:q0 + 512],
                                start=True, stop=True,
                            )
                        if len(pend) >= AV_SKEW:
                            pkt, ppw = pend.pop(0)
                            for h in range(2):
                                nc.tensor.matmul(
                                    po[h][:],
                                    v_t[:, pkt, h * 65:h * 65 + 65],
                                    ppw[:, h * 512:(h + 1) * 512],
                                    start=(pkt == 0), stop=False,
                                )
                        pexp = ppool.tile([128, 1024], BF16, tag="pp")
                        nc.scalar.activation(
                            pexp[:], ps[:],
                            mybir.ActivationFunctionType.Exp, scale=SCALE,
                        )
                        pw = ppool.tile([128, 1024], BF16, tag="pp")
                        ebb = (eb_ts[kt][:, q0:q0 + 512]
                               .unsqueeze(1).to_broadcast([128, 2, 512]))
                        nc.vector.tensor_mul(
                            pw[:].rearrange("p (h q) -> p h q", h=2),
                            pexp[:].rearrange("p (h q) -> p h q", h=2),
                            ebb,
                        )
                        pend.append((kt, pw))
                        # exactly one filler piece per kt iteration: the
                        # kt pipeline is ACT(exp)-bound at ~1.1us/kt while
                        # the PE's own scores+av take ~750ns, so a steady
                        # ~1 piece/iter keeps the PE dense (no HAM
                        # re-throttle) without starving the exp stream
                        drain(2 if len(dq) > 24 else 1)
                    for pkt, ppw in pend:
                        for h in range(2):
                            nc.tensor.matmul(
                                po[h][:],
                                v_t[:, pkt, h * 65:h * 65 + 65],
                                ppw[:, h * 512:(h + 1) * 512],
                                start=False, stop=(pkt == NKT - 1),
                            )
                    nc.scalar.copy(oud0[:, q0:q0 + 512], po0[:])
                    nc.scalar.copy(oud1[:, q0:q0 + 512], po1[:])

                    # ---- end-of-qc hooks --------------------------------
                    dq += [("cluster", t) for t in qc_cluster(b, qc)]
                    st = cl_state[b]
                    if qc == 1:
                        # fire the first-half a2a once its clusters drain
                        dq.append(("fire",
                                   lambda st=st: fire_a2a(*st["a2a"][0])))
                    if qc == 0:
                        if b + 1 < B:
                            load_xt(b + 1)
                    elif qc == 1 and not last:
                        alloc_qkv(b + 1)
                        critn, restn = qkv_chains(b + 1)
                        qkv_rest[b + 1] = restn
                        dq += [("chain", t) for t in critn]
                    elif qc == 2 and not last:
                        dq += [("chain", t)
                               for t in qkv_rest.pop(b + 1)]
                    elif qc == 2 and last:
                        dq += [("proj", t)
                               for t in pending_projs.pop(b - 1)]
                        # the last batch's first-half a2a lands during
                        # qc3: its projection can drain there too
                        dq += [("proj", t) for t in proj_pieces(
                            [(cl_state[b]["a2a"][0][1], 0, HB)],
                            HB, b * TB)]

                # ---- end-of-batch ---------------------------------------
                st = cl_state.pop(b)
                dq.append(("fire", lambda st=st: fire_a2a(*st["a2a"][1])))
                if not last:
                    # b-1's projection drains during b+1's qc0: that zone
                    # is exp-bound with no chain filler left, and proj
                    # pieces read only gat/wp (safe past the xt reload)
                    if b >= 1:
                        dq += [("proj", t)
                               for t in pending_projs.pop(b - 1)]
                    pending_projs[b] = proj_pieces(
                        [(st["a2a"][0][1], 0, HB), (st["a2a"][1][1], HB, HB)],
                        TB, b * TB)
                else:
                    # drain leftovers (finishes the qc3 cluster + h1
                    # fire), then keep the PE warm across the final
                    # collective flight with throwaway matmuls so HAM
                    # stays at 8/8 and the last projection runs fast
                    drain(len(dq))
                    for _ in range(100):
                        pd = ps_m.tile([128, 512], F32, tag="ps_m")
                        nc.tensor.matmul(
                            pd[:], wq_t[:, 0, :], wp_t[:, 0, 0:512],
                            start=True, stop=True,
                        )
                    for t in proj_pieces([(st["a2a"][1][1], 0, HB)],
                                         HB, b * TB + HB):
                        t()

    nc.compile()
    return nc


def _graph():
    global _GRAPH
    if _GRAPH is None:
        _GRAPH = _build()
    return _GRAPH


def _prep_inputs(x, W_qkv, W_proj, b_proj, global_bias):
    x = np.asarray(x, dtype=np.float32)
    W_qkv = np.asarray(W_qkv, dtype=np.float32)
    W_proj = np.asarray(W_proj, dtype=np.float32)
    b_proj = np.asarray(b_proj, dtype=np.float32)
    global_bias = np.asarray(global_bias, dtype=np.float32)

    xt = np.ascontiguousarray(x.reshape(TOK, C).T).astype(BF16_NP)
    wpv = np.ascontiguousarray(W_proj.T).astype(BF16_NP)
    bpv = np.ascontiguousarray(b_proj[:, None])
    ebt = np.exp(global_bias).T  # [k, q]
    ebp = np.ascontiguousarray(
        ebt.reshape(NKT, 128, N).transpose(1, 0, 2)
    ).astype(BF16_NP)

    in_maps = []
    for c in range(N_CORES):
        r0 = c * 128
        wq_c = np.ascontiguousarray(W_qkv[r0:r0 + 128, :].T).astype(BF16_NP)
        wk_c = np.ascontiguousarray(W_qkv[C + r0:C + r0 + 128, :].T).astype(BF16_NP)
        vt = W_qkv[2 * C + r0:2 * C + r0 + 128, :].T  # [C, 128]
        wv_c = np.zeros((C, 130), dtype=np.float32)
        wv_c[:, 0:64] = vt[:, 0:64]
        wv_c[:, 65:129] = vt[:, 64:128]
        in_maps.append({
            "xt": xt,
            "wq": wq_c,
            "wk": wk_c,
            "wv": wv_c.astype(BF16_NP),
            "wp": wpv,
            "bp": bpv,
            "eb": ebp,
        })
    return in_maps


def _assemble(results):
    full = np.empty((TOK, C), dtype=np.float32)
    for c in range(N_CORES):
        o = results[c]["out"].T  # [TSLICE tokens, C]
        for b in range(B):
            full[b * N + c * HB:b * N + (c + 1) * HB, :] = (
                o[b * TB:b * TB + HB, :]
            )
            full[b * N + 1024 + c * HB:b * N + 1024 + (c + 1) * HB, :] = (
                o[b * TB + HB:(b + 1) * TB, :]
            )
    return full.reshape(B, N, C)


def kernel(x, W_qkv, W_proj, b_proj, global_bias):
    nc = _graph()
    in_maps = _prep_inputs(x, W_qkv, W_proj, b_proj, global_bias)
    res = run_bass_kernel_spmd(nc, in_maps, core_ids=list(range(N_CORES)))
    return _assemble(res.results)


def run_profiled(x, W_qkv, W_proj, b_proj, global_bias, **trace_kwargs):
    """Like kernel() but with NTFF profiling; returns (output, results)."""
    nc = _graph()
    in_maps = _prep_inputs(x, W_qkv, W_proj, b_proj, global_bias)
    res = run_bass_kernel_spmd(
        nc, in_maps, core_ids=list(range(N_CORES)), trace=True, **trace_kwargs
    )
    return _assemble(res.results), res

